# revision 1
# baseline (speedup 1.0000x reference)
"""Trainium2 Bass kernel for nn_CustomEmbeddingRegularizer.

Computes  RATE * (sum(x^2) - sum_i mean_{j in nbr(i)} x_i . x_j)
        = RATE * (sum(x^2) - sum_e w_e * (x[src_e] . x[dst_e])),  w_e = 1/deg(src_e)

Distribution: edges sharded 8 ways (contiguous slices of the src-sorted edge
list). Each core gathers its edges' src rows from a per-core 16K-row table
slice (sorted src spans ~N/8 rows) and dst rows from the replicated full
table via int16 dma_gather against four 32K-row base windows (edges are
stable-partitioned by dst window on the host; the per-edge weight travels
with the permutation so ordering never matters). Per-edge dots and the
weighted reduction run on DVE; sum(x^2) of a disjoint N/8 row slice runs on
ACT. Host sums the 8 [128,2] partials.
"""

import numpy as np

import concourse.bacc as bacc
import concourse.bass as bass
import concourse.mybir as mybir
from concourse.tile import TileContext
from concourse.bass_utils import run_bass_kernel_spmd

RATE = 4 * 0.01
N_CORES = 8
P = 128
D = 128
BUCKET = 32768          # int16-addressable row window for the dst gather
SRC_SLICE = 16384       # per-core src-slice rows (covers max src span per shard)
B = 4096                # edges per batch (= one dma_gather)
C = B // P              # edge columns per partition

_CACHE = {}


def _build(N, NB, sched):
    """Compile the SPMD kernel: NB batches, sched[b] = dst bucket id."""
    nc = bacc.Bacc("TRN2", target_bir_lowering=False, num_swdge_queues=4)
    t_table = nc.dram_tensor("table", [N, D], mybir.dt.float32, kind="ExternalInput")
    t_src_slice = nc.dram_tensor("src_slice", [SRC_SLICE, D], mybir.dt.float32,
                                 kind="ExternalInput")
    t_sq_slice = nc.dram_tensor("sq_slice", [N // N_CORES, D], mybir.dt.float32,
                                kind="ExternalInput")
    t_idx_s = nc.dram_tensor("idx_s", [NB, P, B // 16], mybir.dt.int16,
                             kind="ExternalInput")
    t_idx_d = nc.dram_tensor("idx_d", [NB, P, B // 16], mybir.dt.int16,
                             kind="ExternalInput")
    t_w = nc.dram_tensor("w", [NB, P, C], mybir.dt.float32, kind="ExternalInput")
    t_out = nc.dram_tensor("out", [P, 2], mybir.dt.float32, kind="ExternalOutput")

    FSQ = (N // N_CORES) * D // P    # sumsq free elems per partition

    NSQ = 4
    FCH = FSQ // NSQ

    with TileContext(nc) as tc:
        with (
            tc.tile_pool(name="big", bufs=2) as big,
            tc.tile_pool(name="small", bufs=3) as small,
            tc.tile_pool(name="sqp", bufs=2) as sqp,
            tc.tile_pool(name="accp", bufs=1) as accp,
        ):
            acc = accp.tile([P, 1], mybir.dt.float32, tag="acc")
            nc.vector.memset(acc[:], 0.0)
            sq = accp.tile([P, 1], mybir.dt.float32, tag="sq")
            nc.vector.memset(sq[:], 0.0)

            sq_flat = t_sq_slice[:].rearrange("a b -> (a b)").rearrange(
                "(p f) -> p f", p=P)
            for ch in range(NSQ):
                sl_tile = sqp.tile([P, FCH], mybir.dt.float32, tag="sl")
                nc.sync.dma_start(out=sl_tile[:],
                                  in_=sq_flat[:, ch * FCH:(ch + 1) * FCH])
                sq_scratch = sqp.tile([P, FCH], mybir.dt.float32, tag="sqs")
                sqc = sqp.tile([P, 1], mybir.dt.float32, tag="sqc")
                nc.scalar.activation(out=sq_scratch[:], in_=sl_tile[:],
                                     func=mybir.ActivationFunctionType.Square,
                                     accum_out=sqc[:])
                nc.vector.tensor_tensor(out=sq[:], in0=sq[:], in1=sqc[:],
                                        op=mybir.AluOpType.add)

            q = 0
            for b in range(NB):
                base = sched[b] * BUCKET
                dst_src_ap = t_table[base:min(base + BUCKET, N)]

                xs = big.tile([P, C, D], mybir.dt.float32, tag="xs")
                xd = big.tile([P, C, D], mybir.dt.float32, tag="xd")
                prod = big.tile([P, C, D], mybir.dt.float32, tag="prod")
                ist = small.tile([P, B // 16], mybir.dt.int16, tag="ist")
                idt = small.tile([P, B // 16], mybir.dt.int16, tag="idt")
                wt = small.tile([P, C], mybir.dt.float32, tag="wt")
                dots = small.tile([P, C], mybir.dt.float32, tag="dots")
                wd = small.tile([P, C], mybir.dt.float32, tag="wd")
                bs = small.tile([P, 1], mybir.dt.float32, tag="bs")

                nc.sync.dma_start(out=ist[:], in_=t_idx_s[b])
                nc.sync.dma_start(out=idt[:], in_=t_idx_d[b])
                nc.sync.dma_start(out=wt[:], in_=t_w[b])

                # split each gather across two SWDGE queues: the Q7
                # descriptor-generation rate is the bottleneck and queues
                # process in parallel (wrapped idx layout splits cleanly:
                # idx j -> [j%16, j//16], so halves are column ranges)
                H = B // 2
                HC = C // 2
                for half in range(2):
                    cs = slice(half * (H // 16), (half + 1) * (H // 16))
                    nc.gpsimd.dma_gather(
                        out_ap=xs[:, half * HC:(half + 1) * HC, :],
                        in_ap=t_src_slice[:], idxs_ap=ist[:, cs],
                        num_idxs=H, num_idxs_reg=H, elem_size=D,
                        single_packet=False, queue_num=q % 4)
                    q += 1
                for half in range(2):
                    cs = slice(half * (H // 16), (half + 1) * (H // 16))
                    nc.gpsimd.dma_gather(
                        out_ap=xd[:, half * HC:(half + 1) * HC, :],
                        in_ap=dst_src_ap, idxs_ap=idt[:, cs],
                        num_idxs=H, num_idxs_reg=H, elem_size=D,
                        single_packet=False, queue_num=q % 4)
                    q += 1

                nc.vector.tensor_tensor(out=prod[:], in0=xs[:], in1=xd[:],
                                        op=mybir.AluOpType.mult)
                nc.vector.tensor_reduce(out=dots[:], in_=prod[:],
                                        axis=mybir.AxisListType.X,
                                        op=mybir.AluOpType.add)
                nc.vector.tensor_tensor(out=wd[:], in0=dots[:], in1=wt[:],
                                        op=mybir.AluOpType.mult)
                nc.vector.tensor_reduce(out=bs[:], in_=wd[:],
                                        axis=mybir.AxisListType.X,
                                        op=mybir.AluOpType.add)
                nc.vector.tensor_tensor(out=acc[:], in0=acc[:], in1=bs[:],
                                        op=mybir.AluOpType.add)

            out_t = accp.tile([P, 2], mybir.dt.float32, tag="out")
            nc.vector.tensor_copy(out=out_t[:, 0:1], in_=acc[:])
            nc.vector.tensor_copy(out=out_t[:, 1:2], in_=sq[:])
            nc.sync.dma_start(out=t_out[:], in_=out_t[:])
    nc.compile()
    return nc


def _wrap_idx(a):
    """[B] int16 -> [128, B//16] wrapped (j -> [j%16, j//16]) + replicated x8."""
    blk = a.reshape(B // 16, 16).T
    return np.tile(blk, (8, 1))


def kernel(inputs, edge_src, edge_dst):
    x = np.ascontiguousarray(np.asarray(inputs, dtype=np.float32))
    src = np.asarray(edge_src)
    dst = np.asarray(edge_dst)
    N = x.shape[0]
    E = src.shape[0]
    Ec = E // N_CORES
    assert E % N_CORES == 0 and x.shape[1] == D and N % N_CORES == 0

    src32 = src.astype(np.int64)
    dst32 = dst.astype(np.int64)
    deg = np.bincount(src32, minlength=N)
    w_all = (1.0 / np.maximum(deg, 1))[src32].astype(np.float32)

    n_buckets = (N + BUCKET - 1) // BUCKET

    # per-core, per-bucket edge lists (edge order within a core is free: the
    # weight travels with the edge)
    per_core = []
    for k in range(N_CORES):
        lo, hi = k * Ec, (k + 1) * Ec
        s = src32[lo:hi]
        d = dst32[lo:hi]
        w = w_all[lo:hi]
        b0 = int(s.min())
        span = int(s.max()) - b0 + 1
        if span > SRC_SLICE:
            raise ValueError(f"src span {span} exceeds SRC_SLICE {SRC_SLICE}")
        sl = s - b0                      # local src idx
        g = d >> 15                      # dst bucket (32768 = 2^15)
        order = np.argsort(g, kind="stable")
        per_core.append((b0, sl[order], d[order] - (g[order] << 15),
                         w[order], np.bincount(g, minlength=n_buckets)))

    counts = np.stack([pc[4] for pc in per_core])          # [cores, buckets]
    gmax = counts.max(axis=0)                              # padded per-bucket size
    nb_g = [int(-(-int(m) // B)) for m in gmax]            # batches per bucket
    NB = sum(nb_g)
    sched = []
    for gidx, nb in enumerate(nb_g):
        sched += [gidx] * nb

    key = (N, NB, tuple(sched))
    if key not in _CACHE:
        _CACHE[key] = _build(N, NB, sched)
    nc = _CACHE[key]

    in_maps = []
    for k in range(N_CORES):
        b0, sl, dl, w, cnt = per_core[k]
        # assemble padded per-bucket streams in schedule order
        idx_s = np.zeros((NB, B), dtype=np.int16)
        idx_d = np.zeros((NB, B), dtype=np.int16)
        wv = np.zeros((NB, B), dtype=np.float32)
        pos = 0
        bslot = 0
        for gidx, nb in enumerate(nb_g):
            n = int(cnt[gidx])
            seg_s = sl[pos:pos + n]
            seg_d = dl[pos:pos + n]
            seg_w = w[pos:pos + n]
            pos += n
            flat_s = np.zeros(nb * B, dtype=np.int16)
            flat_d = np.zeros(nb * B, dtype=np.int16)
            flat_w = np.zeros(nb * B, dtype=np.float32)
            flat_s[:n] = seg_s
            flat_d[:n] = seg_d
            flat_w[:n] = seg_w
            idx_s[bslot:bslot + nb] = flat_s.reshape(nb, B)
            idx_d[bslot:bslot + nb] = flat_d.reshape(nb, B)
            wv[bslot:bslot + nb] = flat_w.reshape(nb, B)
            bslot += nb

        idx_s_w = np.stack([_wrap_idx(a) for a in idx_s])
        idx_d_w = np.stack([_wrap_idx(a) for a in idx_d])
        # w layout: edge j -> (partition j%128, col j//128)
        w_t = wv.reshape(NB, C, P).transpose(0, 2, 1).copy()

        src_slice = np.zeros((SRC_SLICE, D), dtype=np.float32)
        avail = min(SRC_SLICE, N - b0)
        src_slice[:avail] = x[b0:b0 + avail]
        sq_slice = x[k * (N // N_CORES):(k + 1) * (N // N_CORES)]

        in_maps.append({
            "table": x,
            "src_slice": src_slice,
            "sq_slice": np.ascontiguousarray(sq_slice),
            "idx_s": idx_s_w,
            "idx_d": idx_d_w,
            "w": w_t,
        })

    res = run_bass_kernel_spmd(nc, in_maps, core_ids=list(range(N_CORES)))
    neighbor = 0.0
    sumsq = 0.0
    for k in range(N_CORES):
        out = res.results[k]["out"].astype(np.float64)
        neighbor += out[:, 0].sum()
        sumsq += out[:, 1].sum()
    return np.float32(RATE * (sumsq - neighbor))



# revision 2
# speedup vs baseline: 1.0001x; 1.0001x over previous
"""Trainium2 Bass kernel for nn_CustomEmbeddingRegularizer.

Computes  RATE * (sum(x^2) - sum_i (1/deg_i) * x_i . s_i),
          s_i = sum_{e: src_e = i} x[dst_e]

Design: the gathered dst block xd [128 edges, 128 d] is the matmul
STATIONARY operand (one weight load per block) and the one-hot M is the
moving operand, producing transposed PSUM windows [128 d, slots]. A
block whose edges span multiple src windows gets ONE matmul of width
N = 128*span (bank-aligned runs), instead of one matmul per window.
One-hots are built in batched DVE tensor_tensor is_equal ops (fp32
operands: slot ids up to 512 are not bf16-exact). Gathers are bf16
(256B/row), 4-way queue-split per cell. u = x/deg is shipped transposed.
"""

import numpy as np
import ml_dtypes

import concourse.bacc as bacc
import concourse.bass as bass
import concourse.mybir as mybir
from concourse.tile import TileContext
from concourse.bass_utils import run_bass_kernel_spmd

RATE = 4 * 0.01
N_CORES = 8
P = 128
D = 128
BUCKET = 32768
GW = 16                 # windows per group (PSUM: GW*128 fp32 = 4 banks)
PAD_SLOT = -(10 ** 6)

BF16 = ml_dtypes.bfloat16

_CACHE = {}


def _build(N, NG, nblk, runs, NSUB, TOTCOL, NBLK_MAX, repeat=1, gsplit=4,
           xbufs=3, spkt=False, drp_bufs=1):
    """runs[g][b] = list of (block j, w0_rel, width_in_windows) in emission
    order; the cell's slots columns are grouped by width (1..4), run order
    preserved within each width group."""
    nc = bacc.Bacc("TRN2", target_bir_lowering=False, num_swdge_queues=4)
    t_table = nc.dram_tensor("table", [N, D], mybir.dt.bfloat16,
                             kind="ExternalInput")
    t_u = nc.dram_tensor("u", [P, NG * GW * 128], mybir.dt.bfloat16,
                         kind="ExternalInput")
    t_idx = nc.dram_tensor("idx", [P, TOTCOL], mybir.dt.int16,
                           kind="ExternalInput")
    t_slots = nc.dram_tensor("slots", [P, NSUB], mybir.dt.float32,
                             kind="ExternalInput")
    t_iota = nc.dram_tensor("iota", [P, 512], mybir.dt.float32,
                            kind="ExternalInput")
    t_sq = nc.dram_tensor("sq_slice", [N // N_CORES, D], mybir.dt.float32,
                          kind="ExternalInput")
    t_out = nc.dram_tensor("out", [P, 2], mybir.dt.float32,
                           kind="ExternalOutput")

    FSQ = (N // N_CORES) * D // P
    NSQ = 8
    FCH = FSQ // NSQ

    # per-width max run count per cell (for tile sizing)
    MBNW = [1] * 5
    for g in range(NG):
        for b in range(4):
            for wd in (1, 2, 3, 4):
                n = sum(1 for (_, _, w) in runs[g][b] if w == wd)
                MBNW[wd] = max(MBNW[wd], n)

    with TileContext(nc) as tc:
        with (
            tc.tile_pool(name="const", bufs=1) as const,
            tc.tile_pool(name="accp", bufs=1) as accp,
            tc.tile_pool(name="sqp", bufs=2) as sqp,
            tc.tile_pool(name="idxp", bufs=xbufs) as idxp,
            tc.tile_pool(name="xdp", bufs=xbufs) as xdp,
            tc.tile_pool(name="mp", bufs=2) as mp,
            tc.tile_pool(name="up", bufs=2) as up,
            tc.tile_pool(name="drp", bufs=drp_bufs) as drp,
            tc.tile_pool(name="psump", bufs=2, space="PSUM") as psump,
        ):
            acc = accp.tile([P, 1], mybir.dt.float32, tag="acc")
            nc.vector.memset(acc[:], 0.0)
            sq = accp.tile([P, 1], mybir.dt.float32, tag="sq")
            nc.vector.memset(sq[:], 0.0)

            iota_sb = const.tile([P, 512], mybir.dt.float32, tag="iota")
            nc.sync.dma_start(out=iota_sb[:], in_=t_iota[:])
            slots_sb = const.tile([P, NSUB], mybir.dt.float32, tag="slots")
            nc.sync.dma_start(out=slots_sb[:], in_=t_slots[:])
            mzero = const.tile([P, 128], mybir.dt.bfloat16, tag="mzero")
            nc.vector.memset(mzero[:], 0.0)

            # sum(x^2) on ACT, fp32 (chunks cover FSQ exactly)
            sq_flat = t_sq[:].rearrange("a b -> (a b)").rearrange(
                "(p f) -> p f", p=P)
            off_sq = 0
            for ch in range(NSQ):
                fch = FSQ // NSQ + (1 if ch < FSQ % NSQ else 0)
                sl_tile = sqp.tile([P, FCH + 1], mybir.dt.float32, tag="sl")
                nc.sync.dma_start(out=sl_tile[:, :fch],
                                  in_=sq_flat[:, off_sq:off_sq + fch])
                sq_scratch = sqp.tile([P, FCH + 1], mybir.dt.float32,
                                      tag="sqs")
                sqc = sqp.tile([P, 1], mybir.dt.float32, tag="sqc")
                nc.scalar.activation(out=sq_scratch[:, :fch],
                                     in_=sl_tile[:, :fch],
                                     func=mybir.ActivationFunctionType.Square,
                                     accum_out=sqc[:])
                nc.vector.tensor_tensor(out=sq[:], in0=sq[:], in1=sqc[:],
                                        op=mybir.AluOpType.add)
                off_sq += fch

            def body():
                sub_i = 0
                col_off = 0
                q = 0
                for g in range(NG):
                    pg = psump.tile([P, GW * 128], mybir.dt.float32, tag="pg")
                    for w in range(GW):
                        nc.tensor.matmul(pg[:, w * 128:(w + 1) * 128],
                                         mzero[:], mzero[:],
                                         start=True, stop=False,
                                         skip_group_check=True)
                    for b in range(4):
                        nb = nblk[g][b]
                        if nb == 0:
                            continue
                        ncols = nb * 8
                        idxt = idxp.tile([P, NBLK_MAX * 8], mybir.dt.int16,
                                         tag="idx")
                        nc.sync.dma_start(out=idxt[:, :ncols],
                                          in_=t_idx[:, col_off:col_off + ncols])
                        xd = xdp.tile([P, NBLK_MAX, D], mybir.dt.bfloat16,
                                      tag="xd")
                        base = b * BUCKET
                        in_ap = t_table[base:min(base + BUCKET, N)]
                        bounds = [round(i * nb / gsplit)
                                  for i in range(gsplit + 1)]
                        for i in range(gsplit):
                            c0, c1 = bounds[i], bounds[i + 1]
                            if c1 <= c0:
                                continue
                            nh = c1 - c0
                            nc.gpsimd.dma_gather(
                                out_ap=xd[:, c0:c1, :],
                                in_ap=in_ap,
                                idxs_ap=idxt[:, c0 * 8:c1 * 8],
                                num_idxs=nh * 128, num_idxs_reg=nh * 128,
                                elem_size=D, single_packet=spkt,
                                queue_num=q % 4)
                            q += 1
                        cell = runs[g][b]
                        # batched one-hot builds, grouped by width
                        nw = [0] * 5            # runs per width
                        for (_, _, wd) in cell:
                            nw[wd] += 1
                        mbs = {}
                        off = sub_i
                        for wd in (1, 2, 3, 4):
                            if nw[wd] == 0:
                                continue
                            W = wd * 128
                            mb = mp.tile([P, MBNW[wd], W],
                                         mybir.dt.bfloat16, tag=f"mb{wd}")
                            nc.vector.tensor_tensor(
                                out=mb[:, :nw[wd], :],
                                in0=iota_sb[:, None, :W].broadcast_to(
                                    [P, nw[wd], W]),
                                in1=slots_sb[:, off:off + nw[wd], None]
                                .broadcast_to([P, nw[wd], W]),
                                op=mybir.AluOpType.is_equal)
                            mbs[wd] = mb
                            off += nw[wd]
                        # matmuls in run order
                        cnt = [0] * 5
                        woff = [0] * 5
                        woff_acc = 0
                        for wd in (1, 2, 3, 4):
                            woff[wd] = woff_acc
                            woff_acc += nw[wd]
                        for (j, w0, wd) in cell:
                            W = wd * 128
                            mslice = mbs[wd][:, cnt[wd]:cnt[wd] + 1, :]
                            nc.tensor.matmul(
                                pg[:, w0 * 128:w0 * 128 + W],
                                xd[:, j:j + 1, :], mslice,
                                start=False, stop=False,
                                skip_group_check=True)
                            cnt[wd] += 1
                        sub_i += len(cell)
                        col_off += ncols
                    for w in range(GW):
                        nc.tensor.matmul(pg[:, w * 128:(w + 1) * 128],
                                         mzero[:], mzero[:],
                                         start=False, stop=True,
                                         skip_group_check=True)
                    ut = up.tile([P, GW * 128], mybir.dt.bfloat16, tag="u")
                    nc.sync.dma_start(
                        out=ut[:],
                        in_=t_u[:, g * GW * 128:(g + 1) * GW * 128])
                    tmp = drp.tile([P, GW * 128], mybir.dt.float32, tag="dr")
                    nc.vector.tensor_tensor(out=tmp[:], in0=pg[:], in1=ut[:],
                                            op=mybir.AluOpType.mult)
                    red = drp.tile([P, 1], mybir.dt.float32, tag="red")
                    nc.vector.tensor_reduce(out=red[:], in_=tmp[:],
                                            axis=mybir.AxisListType.X,
                                            op=mybir.AluOpType.add)
                    nc.vector.tensor_tensor(out=acc[:], in0=acc[:],
                                            in1=red[:],
                                            op=mybir.AluOpType.add)

            if repeat == 1:
                body()
            else:
                with tc.For_i(0, repeat, 1):
                    body()

            out_t = accp.tile([P, 2], mybir.dt.float32, tag="out")
            nc.vector.tensor_copy(out=out_t[:, 0:1], in_=acc[:])
            nc.vector.tensor_copy(out=out_t[:, 1:2], in_=sq[:])
            nc.sync.dma_start(out=t_out[:], in_=out_t[:])
    nc.compile()
    return nc


def _wrap_idx(a):
    blk = a.reshape(-1, 16).T
    return np.tile(blk, (8, 1))


def _prep(inputs, edge_src, edge_dst):
    x = np.ascontiguousarray(np.asarray(inputs, dtype=np.float32))
    src = np.asarray(edge_src).astype(np.int64)
    dst = np.asarray(edge_dst).astype(np.int64)
    N = x.shape[0]
    E = src.shape[0]
    Ec = E // N_CORES
    assert E % N_CORES == 0 and x.shape[1] == D and N % N_CORES == 0

    deg = np.bincount(src, minlength=N)
    u_full = (x / np.maximum(deg, 1)[:, None]).astype(BF16)
    table_bf = x.astype(BF16)

    cores = []
    spans = []
    for k in range(N_CORES):
        s = src[k * Ec:(k + 1) * Ec]
        d = dst[k * Ec:(k + 1) * Ec]
        src0 = int(s[0])
        spans.append(int(s[-1]) - src0 + 1)
        cores.append((s, d, src0))
    NW = max((sp + 127) // 128 for sp in spans)
    NG = (NW + GW - 1) // GW

    counts = np.zeros((N_CORES, NG, 4), np.int64)
    streams = []
    for k in range(N_CORES):
        s, d, src0 = cores[k]
        sl = (s - src0)
        g_e = (sl >> 7) >> 4
        b_e = d >> 15
        d_loc = (d - (b_e << 15)).astype(np.int16)
        order = np.lexsort((b_e, g_e))
        key = (g_e * 4 + b_e)
        counts[k] = np.bincount(key, minlength=NG * 4).reshape(NG, 4)
        streams.append((sl[order], d_loc[order]))

    cmax = counts.max(axis=0)
    nblk = np.ceil(cmax / 128).astype(np.int64)
    NBLK_MAX = max(1, int(nblk.max()))

    cum = np.cumsum(counts.reshape(N_CORES, -1), axis=1)
    cell_off = np.zeros((N_CORES, NG * 4), np.int64)
    cell_off[:, 1:] = cum[:, :-1]

    runs = []
    slots_cols = [[] for _ in range(N_CORES)]
    idx_cols = [[] for _ in range(N_CORES)]
    for g in range(NG):
        runs_g = []
        for b in range(4):
            nb = int(nblk[g][b])
            if nb == 0:
                runs_g.append([])
                continue
            padded_sl = np.full((N_CORES, nb * 128), PAD_SLOT, np.int64)
            padded_dl = np.zeros((N_CORES, nb * 128), np.int16)
            for k in range(N_CORES):
                c = int(counts[k, g, b])
                o = int(cell_off[k, g * 4 + b])
                padded_sl[k, :c] = streams[k][0][o:o + c]
                padded_dl[k, :c] = streams[k][1][o:o + c]
            # per-block window unions across cores -> bank-aligned runs
            cell_runs = []
            for j in range(nb):
                wmin, wmax = None, None
                for k in range(N_CORES):
                    c = int(counts[k, g, b])
                    if c <= j * 128:
                        continue
                    lo = j * 128
                    hi = min((j + 1) * 128, c) - 1
                    wf = int(padded_sl[k, lo]) >> 7
                    wl = int(padded_sl[k, hi]) >> 7
                    wmin = wf if wmin is None else min(wmin, wf)
                    wmax = wl if wmax is None else max(wmax, wl)
                if wmin is None:
                    cell_runs.append((j, 0, 1))
                    continue
                w0 = wmin - g * GW
                w1 = wmax - g * GW
                # split [w0..w1] into bank-aligned (4-window) runs
                wcur = w0
                while wcur <= w1:
                    wend = min(w1, (wcur // 4) * 4 + 3)
                    cell_runs.append((j, wcur, wend - wcur + 1))
                    wcur = wend + 1
            runs_g.append(cell_runs)
            # slots columns grouped by width, run order preserved
            for wd in (1, 2, 3, 4):
                for (j, w0, wdi) in cell_runs:
                    if wdi != wd:
                        continue
                    wabs = (g * GW + w0) * 128
                    for k in range(N_CORES):
                        slots_cols[k].append(
                            padded_sl[k, j * 128:(j + 1) * 128] - wabs)
            for k in range(N_CORES):
                idx_cols[k].append(_wrap_idx(padded_dl[k]))
        runs.append(runs_g)

    NSUB = len(slots_cols[0])
    TOTCOL = sum(c.shape[1] for c in idx_cols[0])

    sched_key = (3, N, NG, NSUB, TOTCOL, NBLK_MAX,
                 tuple(tuple(map(int, row)) for row in nblk),
                 tuple(tuple(tuple(r) for r in cell)
                       for gg in runs for cell in gg))

    NWP = NG * GW
    in_maps = []
    iota = np.tile(np.arange(512, dtype=np.float32), (P, 1))
    for k in range(N_CORES):
        _, _, src0 = cores[k]
        rows = src0 + np.arange(NWP * 128)
        valid = rows < N
        u_pad = np.zeros((NWP * 128, D), BF16)
        u_pad[valid] = u_full[rows[valid]]
        # transposed: u_t[p=d, win*128 + slot] = u[src0 + win*128 + slot, p]
        u_t = np.ascontiguousarray(u_pad.T)

        slots_t = np.clip(np.stack(slots_cols[k], axis=1), -32000, 32000)
        slots_t = slots_t.astype(np.float32)
        idx_t = np.concatenate(idx_cols[k], axis=1)

        sq_slice = x[k * (N // N_CORES):(k + 1) * (N // N_CORES)]

        in_maps.append({
            "table": table_bf,
            "u": u_t,
            "idx": np.ascontiguousarray(idx_t),
            "slots": np.ascontiguousarray(slots_t),
            "iota": iota,
            "sq_slice": np.ascontiguousarray(sq_slice),
        })

    build_args = (N, NG,
                  [[int(nblk[g][b]) for b in range(4)] for g in range(NG)],
                  runs, NSUB, TOTCOL, NBLK_MAX)
    return sched_key, build_args, in_maps


def kernel(inputs, edge_src, edge_dst):
    sched_key, build_args, in_maps = _prep(inputs, edge_src, edge_dst)
    if sched_key not in _CACHE:
        _CACHE[sched_key] = _build(*build_args)
    nc = _CACHE[sched_key]

    res = run_bass_kernel_spmd(nc, in_maps, core_ids=list(range(N_CORES)))
    neighbor = 0.0
    sumsq = 0.0
    for k in range(N_CORES):
        out = res.results[k]["out"].astype(np.float64)
        neighbor += out[:, 0].sum()
        sumsq += out[:, 1].sum()
    return np.float32(RATE * (sumsq - neighbor))
